# revision 20
# baseline (speedup 1.0000x reference)
"""nn_Attention multi-head attention on 8 TRN2 NeuronCores.

Sharding: core c handles batch b=c//2 and query-half qh=c%2 (1024 query
tokens). QKV projections run only over the core's OWN 1024 tokens; the
K^T/V halves are exchanged between the two cores of a batch with a
pairwise HBM AllGather (replica groups {2b, 2b+1}), so no projection
work is duplicated. Keys are kept in global batch order on both cores,
so the exchange read-back is program-uniform. The host concatenates the
8 disjoint [1024, 1024] output slices.

Device-side structure (per core):
  - attention in transposed layout S^T = K_h Q_h^T per 128-key tile;
    exp on the Scalar engine straight out of PSUM; AV matmuls run one
    key-tile BEHIND the scores so the exp chain (the ACT engine is
    ~50% of the critical path) is never starved
  - ALL filler work (next pair's QKV projection, V transposes, the
    exchange DMAs/collective, previous pairs' output-projection
    partials, softmax normalization tails) is chopped into pieces of
    at most ~2 matmuls and drained evenly across the 32 key-tile slots
    of each pair's attention, keeping PE insertions between dependent
    score/exp steps short
  - softmax denominators: ones column in the V slots; the reciprocal
    chain reads the denominator row straight from PSUM and is deferred
    several slots so the PE never waits on it
  - output projection accumulates two pairs per PSUM group (bias
    folded into the first batch) into an SBUF f32 accumulator
  - a tiny warm-up AllGather at kernel start absorbs the cross-core
    launch skew so the first real exchange is prompt
"""

import collections
import contextlib

import numpy as np
import orjson

import concourse.bass as bass
import concourse.mybir as mybir
import concourse.tile as tile
from concourse.vector_clock import ScopedClock

# ---------------------------------------------------------------------------
# Workarounds for the walrus build in this container, which accepts at most
# one sync wait per engine instruction (two for EventSemaphore):
#  1. Tile's end-of-kernel drain carries one wait per outstanding semaphore --
#     redistribute over a chain of sync-engine NOPs.
#  2. Tile's scheduler also emits multi-wait body instructions -- split them
#     in the serialized BIR by inserting same-engine NOPs ahead of the
#     offender (engine program order makes the chain equivalent).
# ---------------------------------------------------------------------------


def _patched_drain_and_barrier(self, tick_clock, wait_clock):
    nc = self.nc
    collector = nc.sync.nop()
    wait_clock.add_sem_waits(
        collector.ins, ScopedClock({None: tick_clock.global_clock})
    )
    si = collector.ins.sync_info
    waits = list(si.on_wait or []) if si is not None else []
    if si is not None:
        si.on_wait = waits[:1]
    import bass_rust as _br

    for w in waits[1:]:
        n = nc.sync.nop()
        n.ins.sync_info = _br.SyncInfo(on_wait=[w], on_update=[])

    nc.sync.drain()
    nc.all_engine_barrier()
    assert self.sems is not None
    popped = nc._tile_sem_poison_stack.pop()
    assert popped is self._sem_poison
    nc.clear_and_free_semaphores(list(self.sems.allocated().values()))
    nc.all_engine_barrier()


_WCAPS = {"EventSemaphore": 2}
_wcounter = [0]


def _split_waits_json(bir_bytes: bytes) -> bytes:
    j = orjson.loads(bir_bytes)
    changed_any = False
    for f in j.get("functions", []):
        for b in f.get("blocks", []):
            outl = []
            changed = False
            for ins in b["instructions"]:
                si = ins.get("sync_info")
                waits = (si or {}).get("on_wait") or []
                cap = _WCAPS.get(ins.get("opcode"), 1)
                engine = ins.get("engine")
                if len(waits) > cap and engine and engine != "Unassigned":
                    changed = True
                    extra, keep = waits[:-cap], waits[-cap:]
                    for w in extra:
                        _wcounter[0] += 1
                        outl.append({
                            "name": f"I-wsplit-{_wcounter[0]}",
                            "opcode": "NoOp",
                            "engine": engine,
                            "ins": [],
                            "outs": [],
                            "sync_info": {"on_update": [], "on_wait": [w]},
                        })
                    si["on_wait"] = keep
                outl.append(ins)
            if changed:
                b["instructions"] = outl
                changed_any = True
    return orjson.dumps(j) if changed_any else bir_bytes


def _apply_patches():
    if not getattr(tile.TileContext, "_attn_drain_patched", False):
        tile.TileContext._drain_and_barrier = _patched_drain_and_barrier
        tile.TileContext._attn_drain_patched = True
    if not getattr(bass.Bass, "_attn_wait_split_patched", False):
        orig = bass.Bass.to_json_bytes

        def to_json_bytes(self, *a, **kw):
            return _split_waits_json(orig(self, *a, **kw))

        bass.Bass.to_json_bytes = to_json_bytes
        bass.Bass._attn_wait_split_patched = True


F32 = mybir.dt.float32
BF16 = mybir.dt.bfloat16

C = 1024
H = 16
HD = 64
NK = 2048
NQ = 1024
SCALE = HD ** -0.5
KT_TILES = NK // 128   # 16 key tiles (full)
KT_OWN = NQ // 128     # 8 key tiles computed locally
CT_TILES = C // 128
VSLOT = 65             # 64 v dims + ones column
SLOTW = 2 * VSLOT      # both heads of a pair per key tile
NPAIR = H // 2
BOUNCE_W = NQ + KT_OWN * SLOTW  # 1024 K cols + 1040 V cols


def _insert_pieces(queue, items, min_idx):
    """Insert items into the piece deque at the first non-glued boundary at
    or after min_idx (a glued piece must directly follow its predecessor)."""
    q = list(queue)
    idx = min(min_idx, len(q))
    while idx < len(q) and q[idx][0]:
        idx += 1
    q[idx:idx] = items
    queue.clear()
    queue.extend(q)


def build_nc():
    _apply_patches()
    nc = bass.Bass("TRN2", num_devices=8)
    xt = nc.declare_dram_parameter("xt", [C, NQ], BF16, isOutput=False)
    wqkvt = nc.declare_dram_parameter("wqkvt", [C, 3 * C], BF16, isOutput=False)
    wpt = nc.declare_dram_parameter("wpt", [C, C], BF16, isOutput=False)
    bias = nc.declare_dram_parameter("bias", [1, C], BF16, isOutput=False)
    out = nc.declare_dram_parameter("out", [NQ, C], F32, isOutput=True)

    groups = [[0, 1], [2, 3], [4, 5], [6, 7]]

    with tile.TileContext(nc) as tc:
        with contextlib.ExitStack() as es:
            persist = es.enter_context(tc.tile_pool(name="persist", bufs=1))
            ones = persist.tile([1, 128], BF16, tag="ones")
            nc.vector.memset(ones[:], 1.0)
            ident = persist.tile([128, 128], BF16, tag="ident")
            nc.gpsimd.memset(ident[:], 0.0)
            nc.gpsimd.affine_select(
                out=ident[:], in_=ident[:],
                compare_op=mybir.AluOpType.not_equal, fill=1.0,
                base=0, pattern=[[-1, 128]], channel_multiplier=1,
            )
            bias_sb = persist.tile([1, C], BF16, tag="bias")
            nc.sync.dma_start(out=bias_sb[:], in_=bias[:])

            nts = [persist.tile([128, NQ], BF16, tag=f"nt{ct}", name=f"nt{ct}")
                   for ct in range(CT_TILES)]
            vprimes = [persist.tile([128, KT_TILES * SLOTW], BF16,
                                    tag=f"vp{i}", name=f"vp{i}") for i in range(2)]
            oaccs = [persist.tile([128, C], F32, tag=f"oa{tt}", name=f"oa{tt}")
                     for tt in range(NQ // 128)]
            wp_pool = es.enter_context(tc.tile_pool(name="wp", bufs=CT_TILES))
            wpts = [wp_pool.tile([128, C], BF16, tag="wpt", name=f"wpts{ct}")
                    for ct in range(CT_TILES)]

            dram = es.enter_context(tc.tile_pool(name="dram", bufs=1, space="DRAM"))
            b_ins = [dram.tile([128, BOUNCE_W], BF16, tag=f"bi{p}", name=f"bi{p}")
                     for p in range(NPAIR)]
            b_outs = [dram.tile([2, 128, BOUNCE_W], BF16, tag=f"bo{p}",
                                name=f"bo{p}") for p in range(NPAIR)]
            warm_i = dram.tile([128, 16], BF16, tag="warm_i", name="warm_i")
            warm_o = dram.tile([2, 128, 16], BF16, tag="warm_o", name="warm_o")

            psum_mm = es.enter_context(tc.tile_pool(name="psum_mm", bufs=2, space="PSUM"))
            psum_s = es.enter_context(tc.tile_pool(name="psum_s", bufs=2, space="PSUM"))
            psum_u = es.enter_context(tc.tile_pool(name="psum_u", bufs=1, space="PSUM"))

            # warm-up collective: absorbs cross-core launch skew on the CC
            # path while the prologue computes
            nc.sync.dma_start(out=warm_i[0:1, 0:16], in_=bias[0:1, 0:16])
            nc.gpsimd.collective_compute(
                "AllGather", mybir.AluOpType.bypass, replica_groups=groups,
                ins=[warm_i.opt()], outs=[warm_o.opt()],
            )

            with contextlib.ExitStack() as es_attn:
                xt_pool = es_attn.enter_context(tc.tile_pool(name="xtp", bufs=CT_TILES))
                wq_pool = es_attn.enter_context(tc.tile_pool(name="wq", bufs=3))
                kt_pool = es_attn.enter_context(tc.tile_pool(name="kt", bufs=2))
                qt_pool = es_attn.enter_context(tc.tile_pool(name="qt", bufs=3))
                ko_pool = es_attn.enter_context(tc.tile_pool(name="ko", bufs=2))
                vo_pool = es_attn.enter_context(tc.tile_pool(name="vo", bufs=2))
                vstage_pool = es_attn.enter_context(tc.tile_pool(name="vstage", bufs=2))
                exp_pool = es_attn.enter_context(tc.tile_pool(name="exp", bufs=3))
                rsb_pool = es_attn.enter_context(tc.tile_pool(name="rsb", bufs=2))

                xts = []
                for ct in range(CT_TILES):
                    t = xt_pool.tile([128, NQ], BF16, tag="xt", name=f"xts{ct}")
                    nc.sync.dma_start(out=t[:], in_=xt[ct * 128:(ct + 1) * 128, :])
                    xts.append(t)

                def prepare_qkv(p):
                    """Emit weight DMAs for pair p and return
                    (qt_sb, kt_sb, vp, pieces) where pieces is a list of
                    (glued, fn) closures, each <= ~2 matmuls of PE work."""
                    w_sb = wq_pool.tile([128, 3 * CT_TILES * 128], BF16,
                                        tag="w", name=f"w{p}")
                    for ct in range(CT_TILES):
                        nc.sync.dma_start(
                            out=w_sb[:, ct * 384:(ct + 1) * 384],
                            in_=wqkvt[ct * 128:(ct + 1) * 128,
                                      p * 384:(p + 1) * 384],
                        )

                    def w_slice(m, ct):
                        o = ct * 384 + m * 128
                        return w_sb[:, o:o + 128]

                    qt_sb = qt_pool.tile([128, NQ], BF16, tag="qt", name=f"qt{p}")
                    kt_sb = kt_pool.tile([128, NK], BF16, tag="kt", name=f"kt{p}")
                    kown = ko_pool.tile([128, NQ], BF16, tag="kown", name=f"kown{p}")
                    vown = vo_pool.tile([128, KT_OWN * SLOTW], BF16, tag="vown",
                                        name=f"vown{p}")
                    vp = vprimes[p % 2]
                    pieces = []

                    def chunk_pieces(m, tch, drain):
                        cell = {}
                        out = []
                        for i0 in range(0, CT_TILES, 2):
                            def f(i0=i0):
                                if i0 == 0:
                                    cell['ps'] = psum_mm.tile(
                                        [128, 512], F32, tag="mm", name=f"pc{m}")
                                ps = cell['ps']
                                for ct in (i0, i0 + 1):
                                    nc.tensor.matmul(
                                        ps[:], w_slice(m, ct),
                                        xts[ct][:, tch * 512:(tch + 1) * 512],
                                        start=(ct == 0),
                                        stop=(ct == CT_TILES - 1),
                                    )
                                if i0 + 2 == CT_TILES:
                                    drain(ps, tch)
                            out.append((i0 != 0, f))
                        return out

                    def k_drain(ps, tch):
                        nc.vector.tensor_copy(
                            kown[:, tch * 512:(tch + 1) * 512], ps[:])

                    def q_drain(ps, tch):
                        nc.vector.tensor_copy(
                            qt_sb[:, tch * 512:(tch + 1) * 512], ps[:])

                    vcells = [{}, {}]

                    def v_drain(ps, tch):
                        vs = vstage_pool.tile([128, 512], BF16, tag="vs")
                        nc.vector.tensor_copy(vs[:], ps[:])
                        vcells[tch]['vs'] = vs

                    def v_tr(tch, sub):
                        def f():
                            vs = vcells[tch]['vs']
                            kt_idx = tch * 4 + sub
                            pt = psum_mm.tile([128, 512], BF16, tag="mm",
                                              name="pt")
                            nc.tensor.matmul(
                                pt[:, 0:128],
                                vs[:, sub * 128:(sub + 1) * 128],
                                ident[:], is_transpose=True,
                            )
                            so = kt_idx * SLOTW
                            nc.vector.tensor_copy(
                                vown[:, so:so + HD], pt[:, 0:HD])
                            nc.vector.tensor_copy(
                                vown[:, so + VSLOT:so + VSLOT + HD],
                                pt[:, HD:2 * HD])
                        return f

                    def v_memset():
                        # ones columns for the softmax denominators; the
                        # 64-wide copies in v_tr leave them intact
                        nc.vector.memset(vown[:], 1.0)

                    def bounce():
                        nc.sync.dma_start(out=b_ins[p][:, 0:NQ], in_=kown[:])
                        nc.sync.dma_start(out=b_ins[p][:, NQ:BOUNCE_W],
                                          in_=vown[:])

                    def cc():
                        nc.gpsimd.collective_compute(
                            "AllGather", mybir.AluOpType.bypass,
                            replica_groups=groups,
                            ins=[b_ins[p].opt()], outs=[b_outs[p].opt()],
                        )

                    def read(s):
                        def f():
                            nc.sync.dma_start(
                                out=kt_sb[:, s * NQ:(s + 1) * NQ],
                                in_=b_outs[p][s][:, 0:NQ])
                            nc.sync.dma_start(
                                out=vp[:, s * KT_OWN * SLOTW:
                                       (s + 1) * KT_OWN * SLOTW],
                                in_=b_outs[p][s][:, NQ:BOUNCE_W])
                        return f

                    for tch in range(NQ // 512):
                        pieces += chunk_pieces(1, tch, k_drain)
                    pieces.append((False, v_memset))
                    for tch in range(NQ // 512):
                        pieces += chunk_pieces(2, tch, v_drain)
                        for sub in range(4):
                            pieces.append((False, v_tr(tch, sub)))
                    pieces.append((False, bounce))
                    pieces.append((False, cc))
                    for tch in range(NQ // 512):
                        pieces += chunk_pieces(0, tch, q_drain)
                    # stage B: the read-back, run one full pair later so the
                    # collective has ~60us to complete
                    readbacks = [(False, read(0)), (False, read(1))]
                    return qt_sb, kt_sb, vp, pieces, readbacks

                def proj_batch_pieces(pairs, with_bias):
                    """One PSUM accumulation group per (tt, oc) tile over
                    nts[pairs] (+ bias) added into oaccs."""
                    pieces = []
                    first = with_bias
                    for tt in range(NQ // 128):
                        for oc in range(C // 512):
                            def f(tt=tt, oc=oc):
                                po = psum_mm.tile([128, 512], F32, tag="mm",
                                                  name="pp")
                                n = len(pairs) + (1 if with_bias else 0)
                                i = 0
                                if with_bias:
                                    nc.tensor.matmul(
                                        po[:], ones[0:1, :],
                                        bias_sb[0:1, oc * 512:(oc + 1) * 512],
                                        start=True, stop=(n == 1),
                                    )
                                    i = 1
                                for pr in pairs:
                                    nc.tensor.matmul(
                                        po[:],
                                        nts[pr][:, tt * 128:(tt + 1) * 128],
                                        wpts[pr][:, oc * 512:(oc + 1) * 512],
                                        start=(i == 0), stop=(i == n - 1),
                                    )
                                    i += 1
                                osl = oaccs[tt][:, oc * 512:(oc + 1) * 512]
                                if first:
                                    nc.vector.tensor_copy(osl, po[:])
                                else:
                                    nc.vector.tensor_add(out=osl, in0=osl,
                                                         in1=po[:])
                            pieces.append((False, f))
                    return pieces

                def mk_fin(p, h2, stg, t8):
                    """Deferred softmax-normalization tail for head h2 of
                    pair p: reciprocal of the denominators and nt scaling."""
                    nt = nts[p]
                    rb = h2 * 64
                    cell = {}

                    def fa():
                        r8 = rsb_pool.tile([8, 128], BF16, tag="r8", name="r8")
                        with nc.allow_low_precision("bf16 reciprocal"):
                            nc.vector.reciprocal(r8[:], t8[:])
                        rsb = rsb_pool.tile([1, NQ], BF16, tag="r")
                        nc.gpsimd.dma_start(out=rsb[:], in_=r8[:])
                        nc.vector.tensor_copy(nt[rb:rb + 64, :], stg[0:64, :])
                        cell['rsb'] = rsb

                    def fb():
                        rsb = cell['rsb']
                        for qc in range(NQ // 512):
                            pb = psum_mm.tile([128, 512], F32, tag="mm",
                                              name="pb")
                            nc.tensor.matmul(
                                pb[0:64, :], ones[0:1, 0:64],
                                rsb[0:1, qc * 512:(qc + 1) * 512],
                            )
                            nc.vector.tensor_mul(
                                out=nt[rb:rb + 64, qc * 512:(qc + 1) * 512],
                                in0=nt[rb:rb + 64, qc * 512:(qc + 1) * 512],
                                in1=pb[0:64, :],
                            )
                    return [(False, fa), (False, fb)]

                # prologue: pairs 0 and 1 fully staged (QKV + exchange);
                # pair 0's collective completes while pair 1 computes
                qt_sb, kt_sb, vp_cur, sA0, sB0 = prepare_qkv(0)
                for g, f in sA0:
                    f()
                staged = {}
                nqt, nkt, nvp, sA1, sB1 = prepare_qkv(1)
                for g, f in sA1:
                    f()
                # pair-2 K chunks cover the residual pair-0 collective latency
                nqt2, nkt2, nvp2, sA2, sB2 = prepare_qkv(2)
                for g, f in sA2[:8]:
                    f()
                for g, f in sB0:
                    f()
                staged[1] = (nqt, nkt, nvp, sB1)
                staged[2] = (nqt2, nkt2, nvp2, sB2)
                sA2_rest = sA2[8:]

                pending_fin = []
                for p in range(NPAIR):
                    if p == 1:
                        for ct in range(CT_TILES):
                            nc.sync.dma_start(
                                out=wpts[ct][:],
                                in_=wpt[ct * 128:(ct + 1) * 128, :])
                    queue = collections.deque()
                    if p + 1 < NPAIR:
                        queue.extend(staged[p + 1][3])  # read-back of p+1
                    if p == 0:
                        queue.extend(sA2_rest)
                    elif p + 2 < NPAIR:
                        nqt, nkt, nvp, nsA, nsB = prepare_qkv(p + 2)
                        staged[p + 2] = (nqt, nkt, nvp, nsB)
                        queue.extend(nsA)
                    if pending_fin:
                        _insert_pieces(queue, pending_fin, 6)
                        pending_fin = []
                    if p == 2:
                        queue.extend(proj_batch_pieces([0, 1], True))
                    elif p == 4:
                        queue.extend(proj_batch_pieces([2, 3], False))
                    elif p == 6:
                        queue.extend(proj_batch_pieces([4, 5], False))
                    elif p == 7:
                        queue.extend(proj_batch_pieces([6], False))

                    with nc.named_scope(f"attn{p}"):
                        for h2 in range(2):
                            rb = h2 * 64
                            uacc = psum_u.tile([65, NQ], F32, tag="u",
                                               name=f"uacc{h2}")
                            esbs = {}
                            for kt_idx in range(KT_TILES + 1):
                                if kt_idx < KT_TILES:
                                    ps = psum_s.tile([128, NQ], F32, tag="s",
                                                     name="pss")
                                    ko = kt_idx * 128
                                    for qc in range(NQ // 512):
                                        qsl = slice(qc * 512, (qc + 1) * 512)
                                        nc.tensor.matmul(
                                            ps[:, qsl],
                                            kt_sb[rb:rb + 64, ko:ko + 128],
                                            qt_sb[rb:rb + 64, qsl],
                                        )
                                    esb = exp_pool.tile([128, NQ], BF16,
                                                        tag="e")
                                    nc.scalar.activation(
                                        esb[:], ps[:],
                                        mybir.ActivationFunctionType.Exp,
                                        scale=SCALE)
                                    esbs[kt_idx] = esb
                                if kt_idx >= 1:
                                    kprev = kt_idx - 1
                                    pesb = esbs.pop(kprev)
                                    slot = kprev * SLOTW + h2 * VSLOT
                                    for qc in range(NQ // 512):
                                        qsl = slice(qc * 512, (qc + 1) * 512)
                                        nc.tensor.matmul(
                                            uacc[:, qsl],
                                            vp_cur[:, slot:slot + VSLOT],
                                            pesb[:, qsl],
                                            start=(kprev == 0),
                                            stop=(kprev == KT_TILES - 1),
                                        )
                                if kt_idx < KT_TILES:
                                    slots_left = (2 - h2) * KT_TILES - kt_idx
                                    n = -(-len(queue) // max(1, slots_left))
                                    for _ in range(min(n, len(queue))):
                                        g, f = queue.popleft()
                                        f()

                            # drain the AV accumulator (halves, so the next
                            # head's first AV frees early); denominator row
                            # goes straight from PSUM into the deferred
                            # reciprocal chain
                            stg = rsb_pool.tile([65, NQ], BF16, tag="stg",
                                                name="stg")
                            nc.vector.tensor_copy(stg[:, 0:512],
                                                  uacc[:, 0:512])
                            nc.vector.tensor_copy(stg[:, 512:NQ],
                                                  uacc[:, 512:NQ])
                            t8 = rsb_pool.tile([8, 128], BF16, tag="t8",
                                               name="t8")
                            nc.sync.dma_start(out=t8[:], in_=stg[64:65, :])
                            fin = mk_fin(p, h2, stg, t8)
                            if h2 == 0:
                                _insert_pieces(queue, fin, 6)
                            elif p + 1 < NPAIR:
                                pending_fin = fin
                            else:
                                for g, f in fin:
                                    f()
                    if p + 1 < NPAIR:
                        qt_sb, kt_sb, vp_cur = staged[p + 1][:3]

            with contextlib.ExitStack() as es_proj:
                out_pool = es_proj.enter_context(tc.tile_pool(name="outp", bufs=3))
                with nc.named_scope("proj"):
                    for tt in range(NQ // 128):
                        for oc in range(C // 512):
                            # alternate PSUM pools (attention's scores pool is
                            # free here) to deepen the matmul/add rotation
                            pool = psum_mm if (tt * 2 + oc) % 2 == 0 else psum_s
                            po = pool.tile([128, 512], F32, tag="mm" if pool is psum_mm else "s",
                                           name="po")
                            nc.tensor.matmul(
                                po[:],
                                nts[NPAIR - 1][:, tt * 128:(tt + 1) * 128],
                                wpts[NPAIR - 1][:, oc * 512:(oc + 1) * 512],
                            )
                            ob = out_pool.tile([128, 512], F32, tag="ob")
                            nc.vector.tensor_add(
                                out=ob[:],
                                in0=oaccs[tt][:, oc * 512:(oc + 1) * 512],
                                in1=po[:],
                            )
                            nc.sync.dma_start(
                                out=out[tt * 128:(tt + 1) * 128,
                                        oc * 512:(oc + 1) * 512],
                                in_=ob[:],
                            )
    return nc


def make_in_maps(x, w_qkv, w_proj, b_proj):
    import ml_dtypes
    bf16 = ml_dtypes.bfloat16
    # reorder w_qkv rows [3, pair, 128] and transpose -> [c, (pair, m, d)]
    wq = np.asarray(w_qkv).reshape(3, NPAIR, 128, C)
    wqkvt = np.ascontiguousarray(
        wq.transpose(3, 1, 0, 2).reshape(C, 3 * C).astype(bf16))
    wpt = np.ascontiguousarray(np.asarray(w_proj).T.astype(bf16))
    bias = np.ascontiguousarray(np.asarray(b_proj).reshape(1, C).astype(bf16))
    in_maps = []
    for c in range(8):
        b, qh = c // 2, c % 2
        xown = np.asarray(x)[b, qh * NQ:(qh + 1) * NQ]
        xt = np.ascontiguousarray(xown.T.astype(bf16))
        in_maps.append({"xt": xt, "wqkvt": wqkvt, "wpt": wpt, "bias": bias})
    return in_maps


def assemble_output(results, x_shape):
    B, N, Cm = x_shape
    outp = np.empty((B, N, Cm), dtype=np.float32)
    for c in range(8):
        b, qh = c // 2, c % 2
        outp[b, qh * NQ:(qh + 1) * NQ, :] = results[c]["out"]
    return outp


_nc_cache = []


def kernel(x, w_qkv, w_proj, b_proj):
    from concourse.bass_utils import run_bass_kernel_spmd

    _apply_patches()
    x = np.asarray(x)
    if not _nc_cache:
        _nc_cache.append(build_nc())
    nc = _nc_cache[0]
    in_maps = make_in_maps(x, np.asarray(w_qkv), np.asarray(w_proj),
                           np.asarray(b_proj))
    res = run_bass_kernel_spmd(nc, in_maps, core_ids=list(range(8)))
    return assemble_output(res.results, (4, 2048, 1024)).astype(np.float32)


# revision 28
# speedup vs baseline: 1.1369x; 1.1369x over previous
"""nn_Attention multi-head attention on 8 TRN2 NeuronCores.

Sharding: core c handles batch b=c//2 and query-half qh=c%2 (1024 query
tokens). QKV projections run only over the core's OWN 1024 tokens; the
K^T/V halves are exchanged between the two cores of a batch with a
pairwise HBM AllGather (replica groups {2b, 2b+1}), so no projection
work is duplicated. Keys are kept in global batch order on both cores,
so the exchange read-back is program-uniform. The host concatenates the
8 disjoint [1024, 1024] output slices.

Device-side structure (per core):
  - attention in transposed layout S^T = K_h Q_h^T per 128-key tile;
    exp on the Scalar engine straight out of PSUM; AV matmuls run one
    key-tile BEHIND the scores so the exp chain (the ACT engine is
    ~50% of the critical path) is never starved
  - ALL filler work (next pair's QKV projection, V transposes, the
    exchange DMAs/collective, previous pairs' output-projection
    partials, softmax normalization tails) is chopped into pieces of
    at most ~2 matmuls and drained evenly across the 32 key-tile slots
    of each pair's attention, keeping PE insertions between dependent
    score/exp steps short
  - softmax denominators: ones column in the V slots; the reciprocal
    chain reads the denominator row straight from PSUM and is deferred
    several slots so the PE never waits on it
  - output projection accumulates two pairs per PSUM group (bias
    folded into the first batch) into an SBUF f32 accumulator
  - a tiny warm-up AllGather at kernel start absorbs the cross-core
    launch skew so the first real exchange is prompt
"""

import collections
import contextlib

import numpy as np
import orjson

import concourse.bass as bass
import concourse.mybir as mybir
import concourse.tile as tile
from concourse.vector_clock import ScopedClock

# ---------------------------------------------------------------------------
# Workarounds for the walrus build in this container, which accepts at most
# one sync wait per engine instruction (two for EventSemaphore):
#  1. Tile's end-of-kernel drain carries one wait per outstanding semaphore --
#     redistribute over a chain of sync-engine NOPs.
#  2. Tile's scheduler also emits multi-wait body instructions -- split them
#     in the serialized BIR by inserting same-engine NOPs ahead of the
#     offender (engine program order makes the chain equivalent).
# ---------------------------------------------------------------------------


def _patched_drain_and_barrier(self, tick_clock, wait_clock):
    nc = self.nc
    collector = nc.sync.nop()
    wait_clock.add_sem_waits(
        collector.ins, ScopedClock({None: tick_clock.global_clock})
    )
    si = collector.ins.sync_info
    waits = list(si.on_wait or []) if si is not None else []
    if si is not None:
        si.on_wait = waits[:1]
    import bass_rust as _br

    for w in waits[1:]:
        n = nc.sync.nop()
        n.ins.sync_info = _br.SyncInfo(on_wait=[w], on_update=[])

    nc.sync.drain()
    nc.all_engine_barrier()
    assert self.sems is not None
    popped = nc._tile_sem_poison_stack.pop()
    assert popped is self._sem_poison
    nc.clear_and_free_semaphores(list(self.sems.allocated().values()))
    nc.all_engine_barrier()


_WCAPS = {"EventSemaphore": 2}
_wcounter = [0]


def _split_waits_json(bir_bytes: bytes) -> bytes:
    j = orjson.loads(bir_bytes)
    changed_any = False
    for f in j.get("functions", []):
        for b in f.get("blocks", []):
            outl = []
            changed = False
            for ins in b["instructions"]:
                si = ins.get("sync_info")
                waits = (si or {}).get("on_wait") or []
                cap = _WCAPS.get(ins.get("opcode"), 1)
                engine = ins.get("engine")
                if len(waits) > cap and engine and engine != "Unassigned":
                    changed = True
                    extra, keep = waits[:-cap], waits[-cap:]
                    for w in extra:
                        _wcounter[0] += 1
                        outl.append({
                            "name": f"I-wsplit-{_wcounter[0]}",
                            "opcode": "NoOp",
                            "engine": engine,
                            "ins": [],
                            "outs": [],
                            "sync_info": {"on_update": [], "on_wait": [w]},
                        })
                    si["on_wait"] = keep
                outl.append(ins)
            if changed:
                b["instructions"] = outl
                changed_any = True
    return orjson.dumps(j) if changed_any else bir_bytes


def _apply_patches():
    if not getattr(tile.TileContext, "_attn_drain_patched", False):
        tile.TileContext._drain_and_barrier = _patched_drain_and_barrier
        tile.TileContext._attn_drain_patched = True
    if not getattr(bass.Bass, "_attn_wait_split_patched", False):
        orig = bass.Bass.to_json_bytes

        def to_json_bytes(self, *a, **kw):
            return _split_waits_json(orig(self, *a, **kw))

        bass.Bass.to_json_bytes = to_json_bytes
        bass.Bass._attn_wait_split_patched = True


F32 = mybir.dt.float32
BF16 = mybir.dt.bfloat16

C = 1024
H = 16
HD = 64
NK = 2048
NQ = 1024
SCALE = HD ** -0.5
KT_TILES = NK // 128   # 16 key tiles (full)
KT_OWN = NQ // 128     # 8 key tiles computed locally
CT_TILES = C // 128
VSLOT = 65             # 64 v dims + ones column
SLOTW = 2 * VSLOT      # both heads of a pair per key tile
NPAIR = H // 2
BOUNCE_W = NQ + KT_OWN * SLOTW  # 1024 K cols + 1040 V cols


def _insert_pieces(queue, items, min_idx):
    """Insert items into the piece deque at the first non-glued boundary at
    or after min_idx (a glued piece must directly follow its predecessor)."""
    q = list(queue)
    idx = min(min_idx, len(q))
    while idx < len(q) and q[idx][0]:
        idx += 1
    q[idx:idx] = items
    queue.clear()
    queue.extend(q)


def build_nc():
    _apply_patches()
    nc = bass.Bass("TRN2", num_devices=8)
    xt = nc.declare_dram_parameter("xt", [C, NQ], BF16, isOutput=False)
    wqkvt = nc.declare_dram_parameter("wqkvt", [C, 3 * C], BF16, isOutput=False)
    wpt = nc.declare_dram_parameter("wpt", [C, C], BF16, isOutput=False)
    bias = nc.declare_dram_parameter("bias", [1, C], BF16, isOutput=False)
    out = nc.declare_dram_parameter("out", [NQ, C], F32, isOutput=True)

    groups = [[0, 1], [2, 3], [4, 5], [6, 7]]

    with tile.TileContext(nc) as tc:
        with contextlib.ExitStack() as es:
            persist = es.enter_context(tc.tile_pool(name="persist", bufs=1))
            ones = persist.tile([1, 128], BF16, tag="ones")
            nc.vector.memset(ones[:], 1.0)
            ident = persist.tile([128, 128], BF16, tag="ident")
            nc.gpsimd.memset(ident[:], 0.0)
            nc.gpsimd.affine_select(
                out=ident[:], in_=ident[:],
                compare_op=mybir.AluOpType.not_equal, fill=1.0,
                base=0, pattern=[[-1, 128]], channel_multiplier=1,
            )
            bias_sb = persist.tile([1, C], BF16, tag="bias")
            nc.sync.dma_start(out=bias_sb[:], in_=bias[:])

            nts = [persist.tile([128, NQ], BF16, tag=f"nt{ct}", name=f"nt{ct}")
                   for ct in range(CT_TILES)]
            vprimes = [persist.tile([128, KT_TILES * SLOTW], BF16,
                                    tag=f"vp{i}", name=f"vp{i}") for i in range(2)]
            oaccs = [persist.tile([128, C], F32, tag=f"oa{tt}", name=f"oa{tt}")
                     for tt in range(NQ // 128)]
            wp_pool = es.enter_context(tc.tile_pool(name="wp", bufs=CT_TILES))
            wpts = [wp_pool.tile([128, C], BF16, tag="wpt", name=f"wpts{ct}")
                    for ct in range(CT_TILES)]

            dram = es.enter_context(tc.tile_pool(name="dram", bufs=1, space="DRAM"))
            b_ins = [dram.tile([128, BOUNCE_W], BF16, tag=f"bi{p}", name=f"bi{p}")
                     for p in range(NPAIR)]
            b_outs = [dram.tile([2, 128, BOUNCE_W], BF16, tag=f"bo{p}",
                                name=f"bo{p}") for p in range(NPAIR)]
            warm_i = dram.tile([128, 16], BF16, tag="warm_i", name="warm_i")
            warm_o = dram.tile([2, 128, 16], BF16, tag="warm_o", name="warm_o")

            psum_mm = es.enter_context(tc.tile_pool(name="psum_mm", bufs=2, space="PSUM"))
            psum_s = es.enter_context(tc.tile_pool(name="psum_s", bufs=2, space="PSUM"))
            psum_u = es.enter_context(tc.tile_pool(name="psum_u", bufs=1, space="PSUM"))

            # warm-up collective: absorbs cross-core launch skew on the CC
            # path while the prologue computes
            nc.sync.dma_start(out=warm_i[0:1, 0:16], in_=bias[0:1, 0:16])
            nc.gpsimd.collective_compute(
                "AllGather", mybir.AluOpType.bypass, replica_groups=groups,
                ins=[warm_i.opt()], outs=[warm_o.opt()],
            )

            with contextlib.ExitStack() as es_attn:
                xt_pool = es_attn.enter_context(tc.tile_pool(name="xtp", bufs=CT_TILES))
                wq_pool = es_attn.enter_context(tc.tile_pool(name="wq", bufs=3))
                ktd_pool = es_attn.enter_context(tc.tile_pool(name="ktd", bufs=2))
                qt_pool = es_attn.enter_context(tc.tile_pool(name="qt", bufs=3))
                qtd_pool = es_attn.enter_context(tc.tile_pool(name="qtd", bufs=2))
                ko_pool = es_attn.enter_context(tc.tile_pool(name="ko", bufs=2))
                vo_pool = es_attn.enter_context(tc.tile_pool(name="vo", bufs=2))
                vstage_pool = es_attn.enter_context(tc.tile_pool(name="vstage", bufs=2))
                exp_pool = es_attn.enter_context(tc.tile_pool(name="exp", bufs=4))
                rsb_pool = es_attn.enter_context(tc.tile_pool(name="rsb", bufs=2))

                xts = []
                for ct in range(CT_TILES):
                    t = xt_pool.tile([128, NQ], BF16, tag="xt", name=f"xts{ct}")
                    nc.sync.dma_start(out=t[:], in_=xt[ct * 128:(ct + 1) * 128, :])
                    xts.append(t)

                def prepare_qkv(p):
                    """Emit weight DMAs for pair p and return
                    (qt_sb, kt_sb, vp, pieces) where pieces is a list of
                    (glued, fn) closures, each <= ~2 matmuls of PE work."""
                    w_sb = wq_pool.tile([128, 3 * CT_TILES * 128], BF16,
                                        tag="w", name=f"w{p}")
                    for ct in range(CT_TILES):
                        nc.sync.dma_start(
                            out=w_sb[:, ct * 384:(ct + 1) * 384],
                            in_=wqkvt[ct * 128:(ct + 1) * 128,
                                      p * 384:(p + 1) * 384],
                        )

                    def w_slice(m, ct):
                        o = ct * 384 + m * 128
                        return w_sb[:, o:o + 128]

                    qt_sb = qt_pool.tile([128, NQ], BF16, tag="qt", name=f"qt{p}")
                    # K^T/Q^T duplicated into both partition halves so
                    # consecutive score matmuls alternate PE row groups
                    # (weight-load overlap)
                    ktd = [ktd_pool.tile([128, NK], BF16, tag=f"ktd{h}",
                                         name=f"ktd{p}_{h}") for h in range(2)]
                    qtd = [qtd_pool.tile([128, NQ], BF16, tag=f"qtd{h}",
                                         name=f"qtd{p}_{h}") for h in range(2)]
                    kown = ko_pool.tile([128, NQ], BF16, tag="kown", name=f"kown{p}")
                    vown = vo_pool.tile([128, KT_OWN * SLOTW], BF16, tag="vown",
                                        name=f"vown{p}")
                    vp = vprimes[p % 2]
                    pieces = []

                    def chunk_pieces(m, tch, drain):
                        cell = {}
                        out = []
                        for i0 in range(0, CT_TILES, 2):
                            def f(i0=i0):
                                if i0 == 0:
                                    cell['ps'] = psum_mm.tile(
                                        [128, 512], F32, tag="mm", name=f"pc{m}")
                                ps = cell['ps']
                                for ct in (i0, i0 + 1):
                                    nc.tensor.matmul(
                                        ps[:], w_slice(m, ct),
                                        xts[ct][:, tch * 512:(tch + 1) * 512],
                                        start=(ct == 0),
                                        stop=(ct == CT_TILES - 1),
                                    )
                                if i0 + 2 == CT_TILES:
                                    drain(ps, tch)
                            out.append((i0 != 0, f))
                        return out

                    def k_drain(ps, tch):
                        nc.vector.tensor_copy(
                            kown[:, tch * 512:(tch + 1) * 512], ps[:])

                    def q_drain(ps, tch):
                        nc.vector.tensor_copy(
                            qt_sb[:, tch * 512:(tch + 1) * 512], ps[:])

                    vcells = [{}, {}]

                    def v_drain(ps, tch):
                        vs = vstage_pool.tile([128, 512], BF16, tag="vs")
                        nc.vector.tensor_copy(vs[:], ps[:])
                        vcells[tch]['vs'] = vs

                    def v_tr(tch, sub):
                        def f():
                            vs = vcells[tch]['vs']
                            kt_idx = tch * 4 + sub
                            pt = psum_mm.tile([128, 512], BF16, tag="mm",
                                              name="pt")
                            nc.tensor.matmul(
                                pt[:, 0:128],
                                vs[:, sub * 128:(sub + 1) * 128],
                                ident[:], is_transpose=True,
                            )
                            so = kt_idx * SLOTW
                            nc.vector.tensor_copy(
                                vown[:, so:so + HD], pt[:, 0:HD])
                            nc.vector.tensor_copy(
                                vown[:, so + VSLOT:so + VSLOT + HD],
                                pt[:, HD:2 * HD])
                        return f

                    def v_memset():
                        # ones columns for the softmax denominators; the
                        # 64-wide copies in v_tr leave them intact
                        nc.vector.memset(vown[:], 1.0)

                    def bounce():
                        nc.sync.dma_start(out=b_ins[p][:, 0:NQ], in_=kown[:])
                        nc.sync.dma_start(out=b_ins[p][:, NQ:BOUNCE_W],
                                          in_=vown[:])

                    def cc():
                        nc.gpsimd.collective_compute(
                            "AllGather", mybir.AluOpType.bypass,
                            replica_groups=groups,
                            ins=[b_ins[p].opt()], outs=[b_outs[p].opt()],
                        )

                    def read(s):
                        def f():
                            # the read-back writes the duplicated K^T layout
                            # directly: each head's 64 rows land in both
                            # partition halves
                            for h in range(2):
                                for half in range(2):
                                    nc.sync.dma_start(
                                        out=ktd[h][half * 64:half * 64 + 64,
                                                   s * NQ:(s + 1) * NQ],
                                        in_=b_outs[p][s][h * 64:(h + 1) * 64,
                                                         0:NQ])
                            nc.sync.dma_start(
                                out=vp[:, s * KT_OWN * SLOTW:
                                       (s + 1) * KT_OWN * SLOTW],
                                in_=b_outs[p][s][:, NQ:BOUNCE_W])
                        return f

                    def qdup():
                        for h in range(2):
                            for half in range(2):
                                nc.sync.dma_start(
                                    out=qtd[h][half * 64:half * 64 + 64, :],
                                    in_=qt_sb[h * 64:(h + 1) * 64, :])

                    for tch in range(NQ // 512):
                        pieces += chunk_pieces(1, tch, k_drain)
                    pieces.append((False, v_memset))
                    for tch in range(NQ // 512):
                        pieces += chunk_pieces(2, tch, v_drain)
                        for sub in range(4):
                            pieces.append((False, v_tr(tch, sub)))
                    pieces.append((False, bounce))
                    pieces.append((False, cc))
                    for tch in range(NQ // 512):
                        pieces += chunk_pieces(0, tch, q_drain)
                    # stage B: the read-back + Q duplication, run one full
                    # pair later so the collective has ~60us to complete
                    readbacks = [(False, read(0)), (False, read(1)),
                                 (False, qdup)]
                    return qtd, ktd, vp, pieces, readbacks

                def proj_batch_pieces(pairs, with_bias):
                    """One PSUM accumulation group per (tt, oc) tile over
                    nts[pairs] (+ bias) added into oaccs."""
                    pieces = []
                    first = with_bias
                    for tt in range(NQ // 128):
                        for oc in range(C // 512):
                            def f(tt=tt, oc=oc):
                                po = psum_mm.tile([128, 512], F32, tag="mm",
                                                  name="pp")
                                n = len(pairs) + (1 if with_bias else 0)
                                i = 0
                                if with_bias:
                                    nc.tensor.matmul(
                                        po[:], ones[0:1, :],
                                        bias_sb[0:1, oc * 512:(oc + 1) * 512],
                                        start=True, stop=(n == 1),
                                    )
                                    i = 1
                                for pr in pairs:
                                    nc.tensor.matmul(
                                        po[:],
                                        nts[pr][:, tt * 128:(tt + 1) * 128],
                                        wpts[pr][:, oc * 512:(oc + 1) * 512],
                                        start=(i == 0), stop=(i == n - 1),
                                    )
                                    i += 1
                                osl = oaccs[tt][:, oc * 512:(oc + 1) * 512]
                                if first:
                                    nc.vector.tensor_copy(osl, po[:])
                                else:
                                    nc.vector.tensor_add(out=osl, in0=osl,
                                                         in1=po[:])
                            pieces.append((False, f))
                    return pieces

                def mk_fin(p, h2, stg, t8):
                    """Deferred softmax-normalization tail for head h2 of
                    pair p: reciprocal of the denominators and nt scaling."""
                    nt = nts[p]
                    rb = h2 * 64
                    cell = {}

                    def fa():
                        r8 = rsb_pool.tile([8, 128], BF16, tag="r8", name="r8")
                        with nc.allow_low_precision("bf16 reciprocal"):
                            nc.vector.reciprocal(r8[:], t8[:])
                        rsb = rsb_pool.tile([1, NQ], BF16, tag="r")
                        nc.gpsimd.dma_start(out=rsb[:], in_=r8[:])
                        nc.vector.tensor_copy(nt[rb:rb + 64, :], stg[0:64, :])
                        cell['rsb'] = rsb

                    def fb():
                        rsb = cell['rsb']
                        for qc in range(NQ // 512):
                            pb = psum_mm.tile([128, 512], F32, tag="mm",
                                              name="pb")
                            nc.tensor.matmul(
                                pb[0:64, :], ones[0:1, 0:64],
                                rsb[0:1, qc * 512:(qc + 1) * 512],
                            )
                            nc.vector.tensor_mul(
                                out=nt[rb:rb + 64, qc * 512:(qc + 1) * 512],
                                in0=nt[rb:rb + 64, qc * 512:(qc + 1) * 512],
                                in1=pb[0:64, :],
                            )
                    return [(False, fa), (False, fb)]

                # prologue: pairs 0 and 1 fully staged (QKV + exchange);
                # pair 0's collective completes while pair 1 computes
                qtd_cur, ktd_cur, vp_cur, sA0, sB0 = prepare_qkv(0)
                for g, f in sA0:
                    f()
                staged = {}
                nqt, nkt, nvp, sA1, sB1 = prepare_qkv(1)
                for g, f in sA1:
                    f()
                for g, f in sB0:
                    f()
                staged[1] = (nqt, nkt, nvp, sB1)

                pending_fin = []
                for p in range(NPAIR):
                    if p == 1:
                        for ct in range(CT_TILES):
                            nc.sync.dma_start(
                                out=wpts[ct][:],
                                in_=wpt[ct * 128:(ct + 1) * 128, :])
                    queue = collections.deque()
                    if p + 1 < NPAIR:
                        queue.extend(staged[p + 1][3])  # read-back of p+1
                    if p + 2 < NPAIR:
                        nqt, nkt, nvp, nsA, nsB = prepare_qkv(p + 2)
                        staged[p + 2] = (nqt, nkt, nvp, nsB)
                        queue.extend(nsA)
                    if pending_fin:
                        _insert_pieces(queue, pending_fin, 6)
                        pending_fin = []
                    if p == 2:
                        queue.extend(proj_batch_pieces([0, 1], True))
                    elif p == 4:
                        queue.extend(proj_batch_pieces([2, 3], False))
                    elif p == 6:
                        queue.extend(proj_batch_pieces([4, 5], False))
                    elif p == 7:
                        queue.extend(proj_batch_pieces([6], False))

                    with nc.named_scope(f"attn{p}"):
                        for h2 in range(2):
                            rb = h2 * 64
                            uacc = psum_u.tile([65, NQ], F32, tag="u",
                                               name=f"uacc{h2}")
                            esbs = {}
                            for kt_idx in range(KT_TILES + 1):
                                if kt_idx < KT_TILES:
                                    ps = psum_s.tile([128, NQ], F32, tag="s",
                                                     name="pss")
                                    ko = kt_idx * 128
                                    for qc in range(NQ // 512):
                                        ab = qc * 64
                                        qsl = slice(qc * 512, (qc + 1) * 512)
                                        nc.tensor.matmul(
                                            ps[:, qsl],
                                            ktd_cur[h2][ab:ab + 64,
                                                        ko:ko + 128],
                                            qtd_cur[h2][ab:ab + 64, qsl],
                                        )
                                    esb = exp_pool.tile([128, NQ], BF16,
                                                        tag="e")
                                    nc.scalar.activation(
                                        esb[:], ps[:],
                                        mybir.ActivationFunctionType.Exp,
                                        scale=SCALE)
                                    esbs[kt_idx] = esb
                                if kt_idx >= 1:
                                    kprev = kt_idx - 1
                                    pesb = esbs.pop(kprev)
                                    slot = kprev * SLOTW + h2 * VSLOT
                                    for qc in range(NQ // 512):
                                        qsl = slice(qc * 512, (qc + 1) * 512)
                                        nc.tensor.matmul(
                                            uacc[:, qsl],
                                            vp_cur[:, slot:slot + VSLOT],
                                            pesb[:, qsl],
                                            start=(kprev == 0),
                                            stop=(kprev == KT_TILES - 1),
                                        )
                                if kt_idx < KT_TILES:
                                    slots_left = (2 - h2) * KT_TILES - kt_idx
                                    n = -(-len(queue) // max(1, slots_left))
                                    for _ in range(min(n, len(queue))):
                                        g, f = queue.popleft()
                                        f()

                            # drain the AV accumulator (halves, so the next
                            # head's first AV frees early); denominator row
                            # goes straight from PSUM into the deferred
                            # reciprocal chain
                            stg = rsb_pool.tile([65, NQ], BF16, tag="stg",
                                                name="stg")
                            nc.vector.tensor_copy(stg[:, 0:512],
                                                  uacc[:, 0:512])
                            nc.vector.tensor_copy(stg[:, 512:NQ],
                                                  uacc[:, 512:NQ])
                            t8 = rsb_pool.tile([8, 128], BF16, tag="t8",
                                               name="t8")
                            nc.sync.dma_start(out=t8[:], in_=stg[64:65, :])
                            fin = mk_fin(p, h2, stg, t8)
                            if h2 == 0:
                                _insert_pieces(queue, fin, 6)
                            elif p + 1 < NPAIR:
                                pending_fin = fin
                            else:
                                for g, f in fin:
                                    f()
                    if p + 1 < NPAIR:
                        qtd_cur, ktd_cur, vp_cur = staged[p + 1][:3]

            with contextlib.ExitStack() as es_proj:
                out_pool = es_proj.enter_context(tc.tile_pool(name="outp", bufs=3))
                with nc.named_scope("proj"):
                    for tt in range(NQ // 128):
                        for oc in range(C // 512):
                            po = psum_mm.tile([128, 512], F32, tag="mm", name="po")
                            nc.tensor.matmul(
                                po[:],
                                nts[NPAIR - 1][:, tt * 128:(tt + 1) * 128],
                                wpts[NPAIR - 1][:, oc * 512:(oc + 1) * 512],
                            )
                            ob = out_pool.tile([128, 512], F32, tag="ob")
                            nc.vector.tensor_add(
                                out=ob[:],
                                in0=oaccs[tt][:, oc * 512:(oc + 1) * 512],
                                in1=po[:],
                            )
                            nc.sync.dma_start(
                                out=out[tt * 128:(tt + 1) * 128,
                                        oc * 512:(oc + 1) * 512],
                                in_=ob[:],
                            )
    return nc


def make_in_maps(x, w_qkv, w_proj, b_proj):
    import ml_dtypes
    bf16 = ml_dtypes.bfloat16
    # reorder w_qkv rows [3, pair, 128] and transpose -> [c, (pair, m, d)]
    wq = np.asarray(w_qkv).reshape(3, NPAIR, 128, C)
    wqkvt = np.ascontiguousarray(
        wq.transpose(3, 1, 0, 2).reshape(C, 3 * C).astype(bf16))
    wpt = np.ascontiguousarray(np.asarray(w_proj).T.astype(bf16))
    bias = np.ascontiguousarray(np.asarray(b_proj).reshape(1, C).astype(bf16))
    in_maps = []
    for c in range(8):
        b, qh = c // 2, c % 2
        xown = np.asarray(x)[b, qh * NQ:(qh + 1) * NQ]
        xt = np.ascontiguousarray(xown.T.astype(bf16))
        in_maps.append({"xt": xt, "wqkvt": wqkvt, "wpt": wpt, "bias": bias})
    return in_maps


def assemble_output(results, x_shape):
    B, N, Cm = x_shape
    outp = np.empty((B, N, Cm), dtype=np.float32)
    for c in range(8):
        b, qh = c // 2, c % 2
        outp[b, qh * NQ:(qh + 1) * NQ, :] = results[c]["out"]
    return outp


_nc_cache = []


def kernel(x, w_qkv, w_proj, b_proj):
    from concourse.bass_utils import run_bass_kernel_spmd

    _apply_patches()
    x = np.asarray(x)
    if not _nc_cache:
        _nc_cache.append(build_nc())
    nc = _nc_cache[0]
    in_maps = make_in_maps(x, np.asarray(w_qkv), np.asarray(w_proj),
                           np.asarray(b_proj))
    res = run_bass_kernel_spmd(nc, in_maps, core_ids=list(range(8)))
    return assemble_output(res.results, (4, 2048, 1024)).astype(np.float32)
